# revision 25
# baseline (speedup 1.0000x reference)
"""Trainium2 Bass kernel for AlignmentModule (per-sample cross-attention).

Reference computation (per batch sample b):
    Q = W @ q + b            # (HID, HW)   1x1-conv channel matmul
    K = W @ p + b            # (HID, HW)
    S = Q^T K                # (HW, HW)
    A = softmax(S, axis=-1)
    aligned = V @ A^T        # (C, HW), V = p
    out = concat([q, aligned], channel axis)

Strategy: data-parallel over batch across 8 NeuronCores (2 samples/core).
Projections stream q/p in fp16 (numerically free for the scores; halves
input HBM traffic); the S^T matmul runs on fp32r-precise Q/K. Softmax is
computed on transposed scores S^T (k on partitions): exp with a constant
-40 shift into bf16, column sums via a ones-matmul on TensorE (which also
broadcasts the sum across partitions), then the weights are NORMALIZED
before the AV contraction (w = e * recip, in (0,1]). That puts the
weights in fp8 range, so NK8 of the 8 key blocks run the AV einsum as
fp8e4m3 DoubleRow matmuls (2 key tiles contracted per instruction = 2x
MAC rate; measured 216ns per 512-col instruction, same as one bf16
matmul) and the rest in bf16. V is host-pretransposed: fp8 for the
DoubleRow kbs, bf16 for the rest.

The AV phase runs in two h-passes (all 512-col h=0 output groups, then
all h=1) so the reciprocal+normalize chain for each h hides under ~20us
of PE work instead of stalling the first h=1 group (a PE idle gap also
costs ~3us of half-clock re-ramp). Output is stored bf16 (host upcasts);
the concat with raw query features is pure host-side data movement.
"""

import sys

if "/opt/trn_rl_repo" not in sys.path:
    sys.path.insert(0, "/opt/trn_rl_repo")

import ml_dtypes
import numpy as np

import concourse.bass as bass
import concourse.mybir as mybir
import concourse.tile as tile
from concourse import bacc
from concourse.bass_utils import run_bass_kernel_spmd

import os
NO_WARMUP = bool(int(os.environ.get("KERNEL_NO_WARMUP", "0")))
NK8 = int(os.environ.get("KERNEL_NK8", "6"))  # key blocks on fp8 DoubleRow

B, C, HID, H, W_ = 16, 2048, 256, 32, 32
HW = H * W_            # 1024
NCORES = 8
BS = B // NCORES       # samples per core
P = 128
CT = C // P            # 16 channel tiles
OT = HID // P          # 2 hid blocks
KT = HW // P           # 8 key blocks
NH = 2                 # free-dim halves of HW
NF = HW // NH          # 512 (PSUM bank / fp32 moving-operand max)
NU = NK8 // 2          # DoubleRow pair count
NKB = KT - NK8         # key blocks contracted in bf16

F32 = mybir.dt.float32
F32R = mybir.dt.float32r
F16 = mybir.dt.float16
BF16 = mybir.dt.bfloat16
F8 = mybir.dt.float8e4
DR = mybir.MatmulPerfMode.DoubleRow
EXP_SHIFT = -40.0

_NC_CACHE = None
LAST_RESULTS = None


def _ensure_ntff_hook():
    """Register the axon NTFF profile hook if the image's antenv lacks it.

    Profiling-only plumbing: run_bass_kernel_spmd(trace=True) under axon
    imports antenv.axon_hooks; some images ship antenv without that
    submodule even though the boot shim has the ctypes implementation.
    """
    import types

    try:
        from antenv.axon_hooks import get_axon_ntff_profile_hook  # noqa: F401
        return
    except ImportError:
        pass
    try:
        from trn_agent_boot.trn_boot import _ntff_profile_via_ctypes
    except ImportError:
        return
    hook = _ntff_profile_via_ctypes("/opt/axon/libaxon_pjrt.so")
    mod = types.ModuleType("antenv.axon_hooks")
    mod._hook = hook
    mod.get_axon_ntff_profile_hook = lambda: mod._hook
    mod.set_axon_ntff_profile_hook = lambda h: setattr(mod, "_hook", h)
    sys.modules["antenv.axon_hooks"] = mod
    import antenv

    antenv.axon_hooks = mod


def _build_nc():
    nc = bacc.Bacc(None, target_bir_lowering=False)

    q_d = nc.declare_dram_parameter("q", [BS, C, HW], F16, isOutput=False)
    p_d = nc.declare_dram_parameter("p", [BS, C, HW], F16, isOutput=False)
    pt8_d = nc.declare_dram_parameter("pt8", [BS, NK8 * P, C], F8, isOutput=False)
    if NKB:
        ptb_d = nc.declare_dram_parameter("ptb", [BS, NKB * P, C], BF16, isOutput=False)
    wt_d = nc.declare_dram_parameter("wt", [C, HID], F16, isOutput=False)
    b_d = nc.declare_dram_parameter("b2", [P, OT], F32, isOutput=False)
    out_d = nc.declare_dram_parameter("out", [BS, C, HW], BF16, isOutput=True)

    Ident = mybir.ActivationFunctionType.Identity
    Exp = mybir.ActivationFunctionType.Exp

    with tile.TileContext(nc) as tc:
        with (
            tc.tile_pool(name="const", bufs=1) as const_pool,
            tc.tile_pool(name="xstream", bufs=12) as x_pool,
            tc.tile_pool(name="vt", bufs=2) as vt_pool,
            tc.tile_pool(name="qf", bufs=1) as qf_pool,
            tc.tile_pool(name="kf", bufs=1) as kf_pool,
            tc.tile_pool(name="e", bufs=1) as e_pool,
            tc.tile_pool(name="w", bufs=1) as w_pool,
            tc.tile_pool(name="rb", bufs=2) as rb_pool,
            tc.tile_pool(name="fold", bufs=2) as fold_pool,
            tc.tile_pool(name="ostage", bufs=1) as o_pool,
            tc.tile_pool(name="acc_ps", bufs=1, space="PSUM") as acc_psum,
            tc.tile_pool(name="av_ps", bufs=4, space="PSUM") as av_psum,
        ):
            # PE warm-up first: ~4us of dummy matmuls so the HAM clock
            # gate opens while the first DMAs are still in flight.
            wu_src = const_pool.tile([P, NF], BF16)
            nc.any.memset(wu_src[:], 0.0)
            wu_sink = const_pool.tile([P, 1], F32)
            if not NO_WARMUP:
                wu_ps = av_psum.tile([P, NF], F32, name="avp")
                for i in range(10):
                    nc.tensor.matmul(
                        wu_ps[:],
                        wu_src[:, :P],
                        wu_src[:],
                        start=(i == 0),
                        stop=(i == 9),
                    )
                nc.vector.tensor_copy(wu_sink[:], wu_ps[:, :1])

            wt_r = wt_d.rearrange("(a p) o -> p a o", p=P)
            wt_s = const_pool.tile([P, CT, HID], F16)
            for w4 in range(CT // 4):
                nc.scalar.dma_start(
                    wt_s[:, 4 * w4:4 * (w4 + 1), :], wt_r[:, 4 * w4:4 * (w4 + 1), :]
                )
            b_s = const_pool.tile([P, OT], F32)
            nc.scalar.dma_start(b_s[:], b_d[:])
            ones_s = const_pool.tile([P, P], BF16)
            nc.any.memset(ones_s[:], 1.0)
            shift_s = const_pool.tile([P, 1], F32)
            nc.any.memset(shift_s[:], EXP_SHIFT)

            vt_tiles = {}

            def _emit_vt8(s):
                vt8 = vt_pool.tile([P, NU, 2, C], F8, name="vt8")
                pt8_r = pt8_d[s].rearrange("(u i p) c -> p u i c", u=NU, i=2, p=P)
                for u in range(NU):
                    nc.scalar.dma_start(vt8[:, u:u + 1, :, :], pt8_r[:, u:u + 1, :, :])
                return vt8

            def _emit_vtb(s):
                if not NKB:
                    return None
                vtb = vt_pool.tile([P, NKB, C], BF16, name="vtb")
                ptb_r = ptb_d[s].rearrange("(a p) c -> p a c", p=P)
                for j in range(NKB):
                    nc.scalar.dma_start(vtb[:, j:j + 1, :], ptb_r[:, j:j + 1, :])
                return vtb

            for s in range(BS):
                # V^T tiles arrive via the ACT HWDGE ring. Sample 0's fp8
                # pairs are issued here (they trickle through leftover HBM
                # bandwidth under the projection stream); everything else
                # is issued at the start of the previous sample's scores
                # phase, where the scores+AV window has ~14MB of spare
                # DMA capacity.
                # --- projections: Qf/Kf [o_p, j, hw] = W @ x + b ---
                # t-outer with 4 live PSUM accumulators (j x h); inputs
                # stream as [128, 2, 1024] fp16 pair-row tiles.
                # fp16: fp32r moving operands stream at ~1.6 cyc/row on
                # real HW under engine concurrency (345ns vs 216ns per
                # 512-col S^T matmul); fp16 is full rate and costs +2e-5
                # rel err (verified in sim)
                qf = qf_pool.tile([P, OT, HW], F16)
                kf = kf_pool.tile([P, OT, HW], F16)
                for src, dst in ((q_d, qf), (p_d, kf)):
                    if s == 0 and src is p_d:
                        # s0's fp8 V^T rides the leftover HBM bandwidth under
                        # the p projection stream + scores phase; issuing it
                        # any earlier starves the q stream instead.
                        vt_tiles[0] = (_emit_vt8(0), None)
                    src_r = src[s].rearrange("(a p) f -> p a f", p=P)
                    pj = [
                        [
                            acc_psum.tile([P, NF], F32, name=f"A{2 * j + h}")
                            for h in range(NH)
                        ]
                        for j in range(OT)
                    ]
                    for u in range(CT // 2):
                        xt = x_pool.tile([P, 2, HW], F16, name="xp")
                        nc.sync.dma_start(xt[:], src_r[:, 2 * u:2 * u + 2, :])
                        for du in range(2):
                            t = 2 * u + du
                            for j in range(OT):
                                for h in range(NH):
                                    nc.tensor.matmul(
                                        pj[j][h][:],
                                        wt_s[:, t, j * P:(j + 1) * P],
                                        xt[:, du, h * NF:(h + 1) * NF],
                                        start=(t == 0),
                                        stop=(t == CT - 1),
                                    )
                        # Sample 0's projection is HBM-starved (nothing
                        # earlier to prefetch under): filler matmuls at the
                        # measured stall slots keep the PE clock from
                        # dropping to the half-speed pstate during the
                        # short input-stream waits.
                        nfil = 0
                        if s == 0 and src is q_d:
                            nfil = {2: 2, 3: 6, 4: 6, 5: 4}.get(u, 0)
                        elif s == 0 and src is p_d:
                            nfil = {0: 6, 1: 6, 2: 2}.get(u, 0)
                        if nfil:
                            fil = av_psum.tile([P, NF], F32, name="avp")
                            for i in range(nfil):
                                nc.tensor.matmul(
                                    fil[:],
                                    wu_src[:, :P],
                                    wu_src[:],
                                    start=(i == 0),
                                    stop=(i == nfil - 1),
                                )
                    # h-major so the h=0 chunks S^T needs first evict
                    # first; on DVE so ACT is free to run the exps the
                    # moment the first S^T group lands
                    for h in range(NH):
                        for j in range(OT):
                            nc.vector.tensor_scalar_add(
                                dst[:, j, h * NF:(h + 1) * NF],
                                pj[j][h][:],
                                b_s[:, j:j + 1],
                            )

                # --- scores^T + exp + softmax denominators + normalize ---
                # h-outer; colsum MMs staggered one kb behind the S^T MMs
                # to give the exp ACT slack. After the reciprocal, DVE
                # normalizes the weights (fp8 for kb<NK8, bf16 for the
                # rest); the AV pass ordering gives this chain ~20us of
                # slack so it never stalls the PE.
                if s == 0:
                    vt_tiles[0] = (vt_tiles[0][0], _emit_vtb(0))
                if s + 1 < BS:
                    vt_tiles[s + 1] = (_emit_vt8(s + 1), _emit_vtb(s + 1))
                vt8, vtb = vt_tiles[s]

                e = e_pool.tile([P, KT, HW], BF16)
                w8 = w_pool.tile([P, NU, 2, HW], F8)
                if NKB:
                    wb = w_pool.tile([P, NKB, HW], BF16)
                rb = rb_pool.tile([P, NH, NF], F32)

                def emit_normalize(h, e=e, w8=w8, wb=wb if NKB else None, rb=rb):
                    for kb in range(KT):
                        if kb < NK8:
                            wdst = w8[:, kb // 2, kb % 2, h * NF:(h + 1) * NF]
                        else:
                            wdst = wb[:, kb - NK8, h * NF:(h + 1) * NF]
                        nc.vector.tensor_mul(
                            wdst, e[:, kb, h * NF:(h + 1) * NF], rb[:, h, :]
                        )

                for h in range(NH):
                    smp = acc_psum.tile([P, NF], F32, name="A3")

                    for kb in range(KT):
                        stp = acc_psum.tile([P, NF], F32, name=f"A{kb % 3}")
                        for j in range(OT):
                            nc.tensor.matmul(
                                stp[:],
                                kf[:, j, kb * P:(kb + 1) * P],
                                qf[:, j, h * NF:(h + 1) * NF],
                                start=(j == 0),
                                stop=(j == OT - 1),
                            )
                        nc.scalar.activation(
                            e[:, kb, h * NF:(h + 1) * NF],
                            stp[:],
                            Exp,
                            bias=shift_s[:],
                            scale=1.0,
                        )

                    # Denominators: fold the 8 kb blocks on DVE (3-op
                    # tree), then ONE ones-matmul for the cross-partition
                    # sum + broadcast (replaces 8 staggered colsum MMs;
                    # the recip chain has 10-20us of slack downstream).
                    hs = slice(h * NF, (h + 1) * NF)
                    fA = fold_pool.tile([P, 4, NF], BF16, name="fA")
                    fB = fold_pool.tile([P, 2, NF], BF16, name="fB")
                    fC = fold_pool.tile([P, NF], BF16, name="fC")
                    nc.vector.tensor_add(fA[:], e[:, 0:4, hs], e[:, 4:8, hs])
                    nc.vector.tensor_add(fB[:], fA[:, 0:2, :], fA[:, 2:4, :])
                    nc.vector.tensor_add(fC[:], fB[:, 0, :], fB[:, 1, :])
                    nc.tensor.matmul(
                        smp[:], ones_s[:], fC[:], start=True, stop=True
                    )
                    nc.vector.reciprocal_approx_fast(rb[:, h, :], smp[:])

                    if h == 0:
                        emit_normalize(0)

                # --- aligned[c_p, q] = V @ w (pre-normalized weights);
                # NU fp8 DoubleRow + NKB bf16 matmuls per PSUM group.
                # Two h-passes: all h=0 groups first, then h=1 (out DMAs
                # per cp issue at the end of the h=1 pass). ---
                out_r = out_d[s].rearrange("(a p) f -> p a f", p=P)
                ots = [
                    o_pool.tile([P, 2, HW], BF16, name=f"ot{cp}")
                    for cp in range(CT // 2)
                ]
                for h in range(NH):
                    for cp in range(CT // 2):
                        if h == 0 and cp == 2:
                            # h=1 normalize sits behind the first few AV
                            # evictions in the in-order DVE queue instead of
                            # starving the PSUM rotation at pass-2 start
                            emit_normalize(1)
                        ot = ots[cp]
                        for dc in range(2):
                            cb = 2 * cp + dc
                            avp = av_psum.tile([P, NF], F32, name="avp")
                            for u in range(NU):
                                nc.tensor.matmul(
                                    avp[:],
                                    vt8[:, u, :, cb * P:(cb + 1) * P],
                                    w8[:, u, :, h * NF:(h + 1) * NF],
                                    start=(u == 0),
                                    stop=(NKB == 0 and u == NU - 1),
                                    perf_mode=DR,
                                )
                            for j in range(NKB):
                                nc.tensor.matmul(
                                    avp[:],
                                    vtb[:, j, cb * P:(cb + 1) * P],
                                    wb[:, j, h * NF:(h + 1) * NF],
                                    start=False,
                                    stop=(j == NKB - 1),
                                )
                            if dc == 0:
                                nc.vector.tensor_copy(
                                    ot[:, dc, h * NF:(h + 1) * NF], avp[:]
                                )
                            else:
                                nc.scalar.activation(
                                    ot[:, dc, h * NF:(h + 1) * NF], avp[:], Ident
                                )
                        if h == NH - 1:
                            if s == BS - 1 and cp == CT // 2 - 1:
                                # split the tail DMA so the kernel end is
                                # not gated on one big transfer
                                for dc in range(2):
                                    for hh in range(NH):
                                        nc.scalar.dma_start(
                                            out_r[
                                                :,
                                                2 * cp + dc:2 * cp + dc + 1,
                                                hh * NF:(hh + 1) * NF,
                                            ],
                                            ot[:, dc:dc + 1, hh * NF:(hh + 1) * NF],
                                        )
                            else:
                                nc.scalar.dma_start(
                                    out_r[:, 2 * cp:2 * cp + 2, :], ot[:]
                                )

    nc.compile()
    return nc


def _get_nc():
    global _NC_CACHE
    if _NC_CACHE is None:
        _NC_CACHE = _build_nc()
    return _NC_CACHE


def kernel(query_features, prompt_features, W, b, _profile=False):
    global LAST_RESULTS
    qv = np.asarray(query_features, dtype=np.float32).reshape(B, C, HW)
    pv = np.asarray(prompt_features, dtype=np.float32).reshape(B, C, HW)
    q16 = np.ascontiguousarray(qv).astype(np.float16)
    p16 = np.ascontiguousarray(pv).astype(np.float16)
    pt = np.ascontiguousarray(pv.transpose(0, 2, 1))
    pt8 = pt[:, :NK8 * P, :].astype(ml_dtypes.float8_e4m3)
    ptb = pt[:, NK8 * P:, :].astype(ml_dtypes.bfloat16)
    wt = np.ascontiguousarray(np.asarray(W, dtype=np.float32).T).astype(np.float16)
    b2 = np.ascontiguousarray(np.asarray(b, dtype=np.float32).reshape(OT, P).T)

    if _profile:
        _ensure_ntff_hook()
    nc = _get_nc()
    in_maps = []
    for i in range(NCORES):
        sl = slice(i * BS, (i + 1) * BS)
        m = {"q": q16[sl], "p": p16[sl], "pt8": pt8[sl], "wt": wt, "b2": b2}
        if NKB:
            m["ptb"] = ptb[sl]
        in_maps.append(m)
    res = run_bass_kernel_spmd(
        nc, in_maps, core_ids=list(range(NCORES)), trace=_profile
    )
    LAST_RESULTS = res
    aligned = np.concatenate(
        [np.asarray(r["out"], dtype=np.float32) for r in res.results], axis=0
    )
    aligned = aligned.reshape(B, C, H, W_)
    full = np.concatenate(
        [np.asarray(query_features, dtype=np.float32).reshape(B, C, H, W_), aligned],
        axis=1,
    )
    return full


# revision 26
# speedup vs baseline: 1.0301x; 1.0301x over previous
"""Trainium2 Bass kernel for AlignmentModule (per-sample cross-attention).

Reference computation (per batch sample b):
    Q = W @ q + b            # (HID, HW)   1x1-conv channel matmul
    K = W @ p + b            # (HID, HW)
    S = Q^T K                # (HW, HW)
    A = softmax(S, axis=-1)
    aligned = V @ A^T        # (C, HW), V = p
    out = concat([q, aligned], channel axis)

Strategy: data-parallel over batch across 8 NeuronCores (2 samples/core).
Projections stream q/p in fp16 (numerically free for the scores; halves
input HBM traffic); the S^T matmul runs on fp32r-precise Q/K. Softmax is
computed on transposed scores S^T (k on partitions): exp with a constant
-40 shift into bf16, column sums via a ones-matmul on TensorE (which also
broadcasts the sum across partitions), then the weights are NORMALIZED
before the AV contraction (w = e * recip, in (0,1]). That puts the
weights in fp8 range, so NK8 of the 8 key blocks run the AV einsum as
fp8e4m3 DoubleRow matmuls (2 key tiles contracted per instruction = 2x
MAC rate; measured 216ns per 512-col instruction, same as one bf16
matmul) and the rest in bf16. V is host-pretransposed: fp8 for the
DoubleRow kbs, bf16 for the rest.

The AV phase runs in two h-passes (all 512-col h=0 output groups, then
all h=1) so the reciprocal+normalize chain for each h hides under ~20us
of PE work instead of stalling the first h=1 group (a PE idle gap also
costs ~3us of half-clock re-ramp). Output is stored bf16 (host upcasts);
the concat with raw query features is pure host-side data movement.
"""

import sys

if "/opt/trn_rl_repo" not in sys.path:
    sys.path.insert(0, "/opt/trn_rl_repo")

import ml_dtypes
import numpy as np

import concourse.bass as bass
import concourse.mybir as mybir
import concourse.tile as tile
from concourse import bacc
from concourse.bass_utils import run_bass_kernel_spmd

import os
NO_WARMUP = bool(int(os.environ.get("KERNEL_NO_WARMUP", "0")))
NK8 = int(os.environ.get("KERNEL_NK8", "6"))  # key blocks on fp8 DoubleRow

B, C, HID, H, W_ = 16, 2048, 256, 32, 32
HW = H * W_            # 1024
NCORES = 8
BS = B // NCORES       # samples per core
P = 128
CT = C // P            # 16 channel tiles
OT = HID // P          # 2 hid blocks
KT = HW // P           # 8 key blocks
NH = 2                 # free-dim halves of HW
NF = HW // NH          # 512 (PSUM bank / fp32 moving-operand max)
NU = NK8 // 2          # DoubleRow pair count
NKB = KT - NK8         # key blocks contracted in bf16

F32 = mybir.dt.float32
F32R = mybir.dt.float32r
F16 = mybir.dt.float16
BF16 = mybir.dt.bfloat16
F8 = mybir.dt.float8e4
DR = mybir.MatmulPerfMode.DoubleRow
EXP_SHIFT = -40.0

_NC_CACHE = None
LAST_RESULTS = None


def _ensure_ntff_hook():
    """Register the axon NTFF profile hook if the image's antenv lacks it.

    Profiling-only plumbing: run_bass_kernel_spmd(trace=True) under axon
    imports antenv.axon_hooks; some images ship antenv without that
    submodule even though the boot shim has the ctypes implementation.
    """
    import types

    try:
        from antenv.axon_hooks import get_axon_ntff_profile_hook  # noqa: F401
        return
    except ImportError:
        pass
    try:
        from trn_agent_boot.trn_boot import _ntff_profile_via_ctypes
    except ImportError:
        return
    hook = _ntff_profile_via_ctypes("/opt/axon/libaxon_pjrt.so")
    mod = types.ModuleType("antenv.axon_hooks")
    mod._hook = hook
    mod.get_axon_ntff_profile_hook = lambda: mod._hook
    mod.set_axon_ntff_profile_hook = lambda h: setattr(mod, "_hook", h)
    sys.modules["antenv.axon_hooks"] = mod
    import antenv

    antenv.axon_hooks = mod


def _build_nc():
    nc = bacc.Bacc(None, target_bir_lowering=False)

    q_d = nc.declare_dram_parameter("q", [BS, C, HW], F16, isOutput=False)
    p_d = nc.declare_dram_parameter("p", [BS, C, HW], F16, isOutput=False)
    pt8_d = nc.declare_dram_parameter("pt8", [BS, NK8 * P, C], F8, isOutput=False)
    if NKB:
        ptb_d = nc.declare_dram_parameter("ptb", [BS, NKB * P, C], BF16, isOutput=False)
    wt_d = nc.declare_dram_parameter("wt", [C, HID], F16, isOutput=False)
    b_d = nc.declare_dram_parameter("b2", [P, OT], F32, isOutput=False)
    out_d = nc.declare_dram_parameter("out", [BS, C, HW], BF16, isOutput=True)

    Ident = mybir.ActivationFunctionType.Identity
    Exp = mybir.ActivationFunctionType.Exp

    with tile.TileContext(nc) as tc:
        with (
            tc.tile_pool(name="const", bufs=1) as const_pool,
            tc.tile_pool(name="xstream", bufs=12) as x_pool,
            tc.tile_pool(name="vt", bufs=2) as vt_pool,
            tc.tile_pool(name="qf", bufs=1) as qf_pool,
            tc.tile_pool(name="kf", bufs=1) as kf_pool,
            tc.tile_pool(name="e", bufs=1) as e_pool,
            tc.tile_pool(name="w", bufs=1) as w_pool,
            tc.tile_pool(name="rb", bufs=2) as rb_pool,
            tc.tile_pool(name="fold", bufs=2) as fold_pool,
            tc.tile_pool(name="ostage", bufs=1) as o_pool,
            tc.tile_pool(name="acc_ps", bufs=1, space="PSUM") as acc_psum,
            tc.tile_pool(name="av_ps", bufs=4, space="PSUM") as av_psum,
        ):
            # PE warm-up first: ~4us of dummy matmuls so the HAM clock
            # gate opens while the first DMAs are still in flight.
            wu_src = const_pool.tile([P, NF], BF16)
            nc.any.memset(wu_src[:], 0.0)
            wu_sink = const_pool.tile([P, 1], F32)
            if not NO_WARMUP:
                wu_ps = av_psum.tile([P, NF], F32, name="avp")
                for i in range(10):
                    nc.tensor.matmul(
                        wu_ps[:],
                        wu_src[:, :P],
                        wu_src[:],
                        start=(i == 0),
                        stop=(i == 9),
                    )
                nc.vector.tensor_copy(wu_sink[:], wu_ps[:, :1])

            wt_r = wt_d.rearrange("(a p) o -> p a o", p=P)
            wt_s = const_pool.tile([P, CT, HID], F16)
            for w4 in range(CT // 4):
                nc.scalar.dma_start(
                    wt_s[:, 4 * w4:4 * (w4 + 1), :], wt_r[:, 4 * w4:4 * (w4 + 1), :]
                )
            b_s = const_pool.tile([P, OT], F32)
            nc.scalar.dma_start(b_s[:], b_d[:])
            ones_s = const_pool.tile([P, P], BF16)
            nc.any.memset(ones_s[:], 1.0)
            shift_s = const_pool.tile([P, 1], F32)
            nc.any.memset(shift_s[:], EXP_SHIFT)

            vt_tiles = {}

            def _emit_vt8(s):
                vt8 = vt_pool.tile([P, NU, 2, C], F8, name="vt8")
                pt8_r = pt8_d[s].rearrange("(u i p) c -> p u i c", u=NU, i=2, p=P)
                for u in range(NU):
                    nc.scalar.dma_start(vt8[:, u:u + 1, :, :], pt8_r[:, u:u + 1, :, :])
                return vt8

            def _emit_vtb(s):
                if not NKB:
                    return None
                vtb = vt_pool.tile([P, NKB, C], BF16, name="vtb")
                ptb_r = ptb_d[s].rearrange("(a p) c -> p a c", p=P)
                for j in range(NKB):
                    nc.scalar.dma_start(vtb[:, j:j + 1, :], ptb_r[:, j:j + 1, :])
                return vtb

            for s in range(BS):
                # V^T tiles arrive via the ACT HWDGE ring. Sample 0's fp8
                # pairs are issued here (they trickle through leftover HBM
                # bandwidth under the projection stream); everything else
                # is issued at the start of the previous sample's scores
                # phase, where the scores+AV window has ~14MB of spare
                # DMA capacity.
                # --- projections: Qf/Kf [o_p, j, hw] = W @ x + b ---
                # t-outer with 4 live PSUM accumulators (j x h); inputs
                # stream as [128, 2, 1024] fp16 pair-row tiles.
                # fp16: fp32r moving operands stream at ~1.6 cyc/row on
                # real HW under engine concurrency (345ns vs 216ns per
                # 512-col S^T matmul); fp16 is full rate and costs +2e-5
                # rel err (verified in sim)
                qf = qf_pool.tile([P, OT, HW], F16)
                kf = kf_pool.tile([P, OT, HW], F16)
                for src, dst in ((q_d, qf), (p_d, kf)):
                    if s == 0 and src is p_d:
                        # s0's fp8 V^T rides the leftover HBM bandwidth under
                        # the p projection stream + scores phase; issuing it
                        # any earlier starves the q stream instead.
                        vt_tiles[0] = (_emit_vt8(0), None)
                    src_r = src[s].rearrange("(a p) f -> p a f", p=P)
                    pj = [
                        [
                            acc_psum.tile([P, NF], F32, name=f"A{2 * j + h}")
                            for h in range(NH)
                        ]
                        for j in range(OT)
                    ]
                    for u in range(CT // 2):
                        xt = x_pool.tile([P, 2, HW], F16, name="xp")
                        nc.sync.dma_start(xt[:], src_r[:, 2 * u:2 * u + 2, :])
                        for du in range(2):
                            t = 2 * u + du
                            for j in range(OT):
                                for h in range(NH):
                                    nc.tensor.matmul(
                                        pj[j][h][:],
                                        wt_s[:, t, j * P:(j + 1) * P],
                                        xt[:, du, h * NF:(h + 1) * NF],
                                        start=(t == 0),
                                        stop=(t == CT - 1),
                                    )
                        # Sample 0's projection is HBM-starved (nothing
                        # earlier to prefetch under): filler matmuls at the
                        # measured stall slots keep the PE clock from
                        # dropping to the half-speed pstate during the
                        # short input-stream waits.
                        nfil = 0
                        if s == 0 and src is q_d:
                            nfil = {2: 2, 3: 6, 4: 6, 5: 4}.get(u, 0)
                        elif s == 0 and src is p_d:
                            nfil = {0: 6, 1: 6, 2: 2}.get(u, 0)
                        if nfil:
                            fil = av_psum.tile([P, NF], F32, name="avp")
                            for i in range(nfil):
                                nc.tensor.matmul(
                                    fil[:],
                                    wu_src[:, :P],
                                    wu_src[:],
                                    start=(i == 0),
                                    stop=(i == nfil - 1),
                                )
                    # h-major so the h=0 chunks S^T needs first evict
                    # first; on DVE so ACT is free to run the exps the
                    # moment the first S^T group lands
                    for h in range(NH):
                        for j in range(OT):
                            nc.vector.tensor_scalar_add(
                                dst[:, j, h * NF:(h + 1) * NF],
                                pj[j][h][:],
                                b_s[:, j:j + 1],
                            )

                # --- scores^T + exp + softmax denominators + normalize ---
                # h-outer; colsum MMs staggered one kb behind the S^T MMs
                # to give the exp ACT slack. After the reciprocal, DVE
                # normalizes the weights (fp8 for kb<NK8, bf16 for the
                # rest); the AV pass ordering gives this chain ~20us of
                # slack so it never stalls the PE.
                if s == 0:
                    vt_tiles[0] = (vt_tiles[0][0], _emit_vtb(0))
                if s + 1 < BS:
                    vt_tiles[s + 1] = (_emit_vt8(s + 1), _emit_vtb(s + 1))
                vt8, vtb = vt_tiles[s]

                e = e_pool.tile([P, KT, HW], BF16)
                w8 = w_pool.tile([P, NU, 2, HW], F8)
                if NKB:
                    wb = w_pool.tile([P, NKB, HW], BF16)
                rb = rb_pool.tile([P, NH, NF], F32)

                def emit_normalize(h, e=e, w8=w8, wb=wb if NKB else None, rb=rb):
                    for kb in range(KT):
                        if kb < NK8:
                            wdst = w8[:, kb // 2, kb % 2, h * NF:(h + 1) * NF]
                        else:
                            wdst = wb[:, kb - NK8, h * NF:(h + 1) * NF]
                        nc.vector.tensor_mul(
                            wdst, e[:, kb, h * NF:(h + 1) * NF], rb[:, h, :]
                        )

                for h in range(NH):
                    smp = acc_psum.tile([P, NF], F32, name="A3")

                    for kb in range(KT):
                        stp = acc_psum.tile([P, NF], F32, name=f"A{kb % 3}")
                        for j in range(OT):
                            nc.tensor.matmul(
                                stp[:],
                                kf[:, j, kb * P:(kb + 1) * P],
                                qf[:, j, h * NF:(h + 1) * NF],
                                start=(j == 0),
                                stop=(j == OT - 1),
                            )
                        nc.scalar.activation(
                            e[:, kb, h * NF:(h + 1) * NF],
                            stp[:],
                            Exp,
                            bias=shift_s[:],
                            scale=1.0,
                        )

                    # Denominators: fold the 8 kb blocks on DVE (3-op
                    # tree), then ONE ones-matmul for the cross-partition
                    # sum + broadcast (replaces 8 staggered colsum MMs;
                    # the recip chain has 10-20us of slack downstream).
                    hs = slice(h * NF, (h + 1) * NF)
                    fA = fold_pool.tile([P, 4, NF], BF16, name="fA")
                    fB = fold_pool.tile([P, 2, NF], BF16, name="fB")
                    fC = fold_pool.tile([P, NF], BF16, name="fC")
                    nc.vector.tensor_add(fA[:], e[:, 0:4, hs], e[:, 4:8, hs])
                    nc.vector.tensor_add(fB[:], fA[:, 0:2, :], fA[:, 2:4, :])
                    nc.vector.tensor_add(fC[:], fB[:, 0, :], fB[:, 1, :])
                    nc.tensor.matmul(
                        smp[:], ones_s[:], fC[:], start=True, stop=True
                    )
                    nc.vector.reciprocal_approx_fast(rb[:, h, :], smp[:])

                    emit_normalize(h)

                # --- aligned[c_p, q] = V @ w (pre-normalized weights);
                # NU fp8 DoubleRow + NKB bf16 matmuls per PSUM group.
                # Two h-passes: all h=0 groups first, then h=1 (out DMAs
                # per cp issue at the end of the h=1 pass). ---
                out_r = out_d[s].rearrange("(a p) f -> p a f", p=P)
                ots = [
                    o_pool.tile([P, 2, HW], BF16, name=f"ot{cp}")
                    for cp in range(CT // 2)
                ]
                for h in range(NH):
                    for cp in range(CT // 2):
                        ot = ots[cp]
                        for dc in range(2):
                            cb = 2 * cp + dc
                            avp = av_psum.tile([P, NF], F32, name="avp")
                            for u in range(NU):
                                nc.tensor.matmul(
                                    avp[:],
                                    vt8[:, u, :, cb * P:(cb + 1) * P],
                                    w8[:, u, :, h * NF:(h + 1) * NF],
                                    start=(u == 0),
                                    stop=(NKB == 0 and u == NU - 1),
                                    perf_mode=DR,
                                )
                            for j in range(NKB):
                                nc.tensor.matmul(
                                    avp[:],
                                    vtb[:, j, cb * P:(cb + 1) * P],
                                    wb[:, j, h * NF:(h + 1) * NF],
                                    start=False,
                                    stop=(j == NKB - 1),
                                )
                            if dc == 0:
                                nc.vector.tensor_copy(
                                    ot[:, dc, h * NF:(h + 1) * NF], avp[:]
                                )
                            else:
                                nc.scalar.activation(
                                    ot[:, dc, h * NF:(h + 1) * NF], avp[:], Ident
                                )
                        if h == NH - 1:
                            if s == BS - 1 and cp == CT // 2 - 1:
                                # split the tail DMA so the kernel end is
                                # not gated on one big transfer
                                for dc in range(2):
                                    for hh in range(NH):
                                        nc.scalar.dma_start(
                                            out_r[
                                                :,
                                                2 * cp + dc:2 * cp + dc + 1,
                                                hh * NF:(hh + 1) * NF,
                                            ],
                                            ot[:, dc:dc + 1, hh * NF:(hh + 1) * NF],
                                        )
                            else:
                                nc.scalar.dma_start(
                                    out_r[:, 2 * cp:2 * cp + 2, :], ot[:]
                                )

    nc.compile()
    return nc


def _get_nc():
    global _NC_CACHE
    if _NC_CACHE is None:
        _NC_CACHE = _build_nc()
    return _NC_CACHE


def kernel(query_features, prompt_features, W, b, _profile=False):
    global LAST_RESULTS
    qv = np.asarray(query_features, dtype=np.float32).reshape(B, C, HW)
    pv = np.asarray(prompt_features, dtype=np.float32).reshape(B, C, HW)
    q16 = np.ascontiguousarray(qv).astype(np.float16)
    p16 = np.ascontiguousarray(pv).astype(np.float16)
    pt = np.ascontiguousarray(pv.transpose(0, 2, 1))
    pt8 = pt[:, :NK8 * P, :].astype(ml_dtypes.float8_e4m3)
    ptb = pt[:, NK8 * P:, :].astype(ml_dtypes.bfloat16)
    wt = np.ascontiguousarray(np.asarray(W, dtype=np.float32).T).astype(np.float16)
    b2 = np.ascontiguousarray(np.asarray(b, dtype=np.float32).reshape(OT, P).T)

    if _profile:
        _ensure_ntff_hook()
    nc = _get_nc()
    in_maps = []
    for i in range(NCORES):
        sl = slice(i * BS, (i + 1) * BS)
        m = {"q": q16[sl], "p": p16[sl], "pt8": pt8[sl], "wt": wt, "b2": b2}
        if NKB:
            m["ptb"] = ptb[sl]
        in_maps.append(m)
    res = run_bass_kernel_spmd(
        nc, in_maps, core_ids=list(range(NCORES)), trace=_profile
    )
    LAST_RESULTS = res
    aligned = np.concatenate(
        [np.asarray(r["out"], dtype=np.float32) for r in res.results], axis=0
    )
    aligned = aligned.reshape(B, C, H, W_)
    full = np.concatenate(
        [np.asarray(query_features, dtype=np.float32).reshape(B, C, H, W_), aligned],
        axis=1,
    )
    return full
